# revision 38
# baseline (speedup 1.0000x reference)
"""Pipelined Trainium2 Bass kernel for the 12-layer LSTM decoder (v2).

Topology: 4 pipeline stages x 2 batch groups over 8 cores.
  core ci: stage s = ci % 4 (layers 3s..3s+2, weights SBUF-resident),
           group g = ci // 4 (batch rows 32g..32g+31).
Chunked pipeline over T: C = T/TC chunks; iteration k is a global slot;
stage s computes chunk c = k - s.  Uniform SPMD: warm-up / drain slots
compute on finite garbage (zero inputs), selected away by per-core 0/1
masks carried in the input map.

v2 changes vs baseline:
  - gates accumulate directly in PSUM: per (layer, quarter) one 2-bank
    PSUM tile, written by bias ident-MMs (start=True), then input MMs
    (W_ih@x), then per-step recurrence MMs (W_hh@h); sigma/tanh read
    PSUM directly.  Kills the per-(m) bias ACTs and per-step ident MMs.
  - W_ih/W_hh stored fp8e3m4 scaled x32 (stationary operand only; FWL
    loads 4 elem/cycle -> half the LDWEIGHTS cost of bf16).  Descale by
    1/32 via the free ACT scale port at sigma/tanh.  W_out stays bf16.
  - AllGather handoff at quarter-chunk granularity (4/slot) so the next
    slot's stage input is ready before this slot ends.
  - x-select and state-carry muxes moved from ACT to DVE tensor_scalar.
"""

import os
import sys

import numpy as np

for _p in ("/opt/trn_rl_repo",):
    if os.path.isdir(_p) and _p not in sys.path:
        sys.path.insert(0, _p)

import ml_dtypes  # noqa: E402

import concourse.bass as bass  # noqa: E402
import concourse.mybir as mybir  # noqa: E402
import concourse.tile as tile  # noqa: E402
from concourse.bass_utils import run_bass_kernel_spmd  # noqa: E402

# ---------------------------------------------------------------- constants
L = 12
B = 64
T = 128
E = 512
H = 512
V = 1000
BOS_ID = 1

NCORES = 8
S = 4                     # pipeline stages
NGRP = 2                  # batch groups
LPS = L // S              # layers per stage = 3
BL = B // NGRP            # batch rows per core = 32
KT = H // 128             # K tiles per 512 contraction = 4
MT = (4 * H) // 128       # M tiles over gate rows = 16
VP = 1024
VMT = VP // 128           # = 8
VPC = VMT // S            # V-tiles per core = 2
TC = 8                    # timesteps per chunk
C = T // TC               # chunks = 16
NITER = C + S - 1         # pipeline slots = 19 (last chunk's logits are
                          # computed locally on stage-3 cores, no drain slot)
NQ = 4                    # quarters per chunk
QS = TC // NQ             # steps per quarter = 2

SW = KT * BL              # per-step h width = 128
GW = MT * BL              # per-step gate width = 512
PW = QS * GW              # psum tile per (l, quarter) = 1024 (2 banks)
XW = TC * KT * BL         # x-chunk width = 1024
XQ = QS * KT * BL         # quarter x width = 256
WSCALE = 32.0             # fp8 weight pre-scale

F32 = mybir.dt.float32
BF16 = mybir.dt.bfloat16
FP8 = mybir.dt.float8e3
BF16_NP = ml_dtypes.bfloat16
FP8_NP = ml_dtypes.float8_e3m4
AF = mybir.ActivationFunctionType
ALU = mybir.AluOpType


# ------------------------------------------------------------- host packing
def _pack_w_stack(w_stack, mt, np_dtype):
    w = np.asarray(w_stack, np.float32)
    squeeze = w.ndim == 2
    if squeeze:
        w = w[None]
    n = w.shape[0]
    kt = w.shape[2] // 128
    out = (
        w.reshape(n, mt, 128, kt, 128)
        .transpose(0, 4, 3, 1, 2)
        .reshape(n, 128, kt * mt * 128)
        .astype(np_dtype)
    )
    return out[0] if squeeze else out


_GATE_PERM = np.concatenate(
    [
        np.arange(0, H),
        np.arange(H, 2 * H),
        np.arange(3 * H, 4 * H),
        np.arange(2 * H, 3 * H),
    ]
)


def _prep_core(inputs, ci):
    st = ci % S
    g = ci // S
    lsl = slice(st * LPS, (st + 1) * LPS)
    bs = slice(g * BL, (g + 1) * BL)

    W_ih = np.asarray(inputs["W_ih"], np.float32)[lsl][:, _GATE_PERM, :]
    W_hh = np.asarray(inputs["W_hh"], np.float32)[lsl][:, _GATE_PERM, :]
    b = (
        np.asarray(inputs["b_ih"], np.float32)
        + np.asarray(inputs["b_hh"], np.float32)
    )[lsl][:, _GATE_PERM]
    wih = _pack_w_stack(W_ih * WSCALE, MT, FP8_NP)            # [3,128,8192]
    whh = _pack_w_stack(W_hh * WSCALE, MT, FP8_NP)

    # bias broadcast tile matching the psum bank layout:
    # per (l): [128, PW] with flat free = (m16, t2, b32); value = WSCALE*b
    bm = (b * WSCALE).reshape(LPS, MT, 128)                   # [3, m, p]
    biasbc = np.broadcast_to(
        bm.transpose(0, 2, 1)[:, :, :, None, None],           # [3,p,m,1,1]
        (LPS, 128, MT, QS, BL),
    ).reshape(LPS, 128, PW).astype(BF16_NP)

    # x0 embeddings (stage 0 only; zeros elsewhere)
    if st == 0:
        tok = np.asarray(inputs["target_token_ids"])
        tokens = np.concatenate(
            [np.full((B, 1), BOS_ID, tok.dtype), tok[:, :-1]], axis=1
        )
        emb = np.asarray(inputs["embed_table"], np.float32)
        e = emb[tokens[bs]]                                   # [32, T, 512]
        x0 = (
            e.reshape(BL, T, KT, 128)
            .transpose(3, 1, 2, 0)
            .reshape(128, T * KT * BL)
            .astype(BF16_NP)
        )
    else:
        x0 = np.zeros((128, T * KT * BL), BF16_NP)

    h0 = np.asarray(inputs["h0"], np.float32)[bs]
    c0 = np.asarray(inputs["c0"], np.float32)[bs]
    h0T = h0.reshape(BL, KT, 128).transpose(2, 1, 0).reshape(128, SW)
    c0T = c0.reshape(BL, KT, 128).transpose(2, 1, 0).reshape(128, SW)

    # output projection slice: V-tiles (2*st, 2*st+1)
    Wo = np.zeros((VP, H), np.float32)
    Wo[:V] = np.asarray(inputs["W_out"], np.float32)
    wo_rows = Wo[st * VPC * 128:(st + 1) * VPC * 128]         # [256, 512]
    wouts = _pack_w_stack(wo_rows, VPC, BF16_NP)              # [128, KT*2*128]
    bo = np.zeros((VP,), np.float32)
    bo[:V] = np.asarray(inputs["b_out"], np.float32)
    bouts = bo.reshape(VMT, 128).T[:, st * VPC:(st + 1) * VPC].copy()

    # full output projection, used only by stage-3 cores to produce the
    # final chunk's logits locally (no AllGather round trip)
    if st == S - 1:
        wof = _pack_w_stack(Wo, VMT, BF16_NP)                 # [128, KT*8*128]
        bof = bo.reshape(VMT, 128).T.copy()                   # [128, 8]
    else:
        wof = np.zeros((128, KT * VMT * 128), BF16_NP)
        bof = np.zeros((128, VMT), np.float32)

    # role masks
    selw = np.zeros((128, 4), np.float32)
    if st > 0:
        selw[:, st - 1] = 1.0
    smask = np.zeros((128, NITER), np.float32)
    smask[:, st + 1:] = 1.0                                   # carry iff k > st
    ismask = 1.0 - smask

    return {
        "wih": np.ascontiguousarray(wih),
        "whh": np.ascontiguousarray(whh),
        "biasbc": np.ascontiguousarray(biasbc),
        "x0": np.ascontiguousarray(x0),
        "h0T": np.ascontiguousarray(h0T.astype(BF16_NP)),
        "c0T": np.ascontiguousarray(c0T.astype(np.float32)),
        "wouts": np.ascontiguousarray(wouts),
        "bouts": np.ascontiguousarray(bouts),
        "wof": np.ascontiguousarray(wof),
        "bof": np.ascontiguousarray(bof),
        "selw": selw,
        "smask": smask,
        "ismask": ismask,
        "identq": np.ascontiguousarray(np.eye(128, dtype=FP8_NP)),
    }


# ------------------------------------------------------------ device kernel
_WAIT_LIMITS = {}


def _split_excess_waits(nc, default_limit=1):
    import bass_rust as _br

    n_split = 0
    for fn in nc.m.functions:
        for bb in fn.blocks:
            insts = list(bb.instructions)
            out = []
            changed = False
            for inst in insts:
                tname = type(inst).__name__
                limit = _WAIT_LIMITS.get(tname, default_limit)
                si = getattr(inst, "sync_info", None)
                ow = list(si.on_wait) if si is not None and si.on_wait else []
                if limit is not None and len(ow) > limit:
                    keep = ow[-limit:] if limit else []
                    rest = ow[:len(ow) - limit]
                    for j, w in enumerate(rest):
                        ev = mybir.InstEventSemaphore(
                            name=f"{inst.name}_wsplit{j}"
                        )
                        ev.engine = inst.engine
                        ev.sync_info = _br.SyncInfo(on_wait=[w], on_update=[])
                        out.append(ev)
                        n_split += 1
                    inst.sync_info = _br.SyncInfo(
                        on_wait=keep, on_update=list(si.on_update or [])
                    )
                    changed = True
                out.append(inst)
            if changed:
                bb.instructions = out
    return n_split


def _build_program():
    nc = bass.Bass(
        "TRN2", target_bir_lowering=False, debug=False, enable_asserts=False
    )

    WQ = KT * MT * 128
    wih_d = nc.dram_tensor("wih", [LPS, 128, WQ], FP8,
                           kind="ExternalInput").ap()
    whh_d = nc.dram_tensor("whh", [LPS, 128, WQ], FP8,
                           kind="ExternalInput").ap()
    biasbc_d = nc.dram_tensor("biasbc", [LPS, 128, PW], BF16,
                              kind="ExternalInput").ap()
    x0_d = nc.dram_tensor("x0", [128, T * KT * BL], BF16,
                          kind="ExternalInput").ap()
    h0_d = nc.dram_tensor("h0T", [128, SW], BF16, kind="ExternalInput").ap()
    c0_d = nc.dram_tensor("c0T", [128, SW], F32, kind="ExternalInput").ap()
    wo_d = nc.dram_tensor("wouts", [128, KT * VPC * 128], BF16,
                          kind="ExternalInput").ap()
    bo_d = nc.dram_tensor("bouts", [128, VPC], F32, kind="ExternalInput").ap()
    wof_d = nc.dram_tensor("wof", [128, KT * VMT * 128], BF16,
                           kind="ExternalInput").ap()
    bof_d = nc.dram_tensor("bof", [128, VMT], F32, kind="ExternalInput").ap()
    selw_d = nc.dram_tensor("selw", [128, 4], F32, kind="ExternalInput").ap()
    sm_d = nc.dram_tensor("smask", [128, NITER], F32,
                          kind="ExternalInput").ap()
    ism_d = nc.dram_tensor("ismask", [128, NITER], F32,
                           kind="ExternalInput").ap()
    identq_d = nc.dram_tensor("identq", [128, 128], FP8,
                              kind="ExternalInput").ap()
    lg_d = nc.dram_tensor("lg", [VPC, 128, (C + 1) * TC * BL], F32,
                          kind="ExternalOutput").ap()
    lgf_d = nc.dram_tensor("lgf", [VMT, 128, TC * BL], F32,
                           kind="ExternalOutput").ap()

    RG = [[0, 1, 2, 3], [4, 5, 6, 7]]

    with tile.TileContext(nc) as tc:
        with (
            tc.tile_pool(name="const", bufs=1) as cpool,
            tc.tile_pool(name="xpool", bufs=2) as xpool,
            tc.tile_pool(name="ccr", bufs=2) as rpool,
            tc.tile_pool(name="spool", bufs=8) as spool,
            tc.tile_pool(name="gpool", bufs=6) as gpool,
            tc.tile_pool(name="gps", bufs=1, space="PSUM") as gps,
            tc.tile_pool(name="lps", bufs=2, space="PSUM") as lps,
            tc.tile_pool(name="dram", bufs=2, space="DRAM") as dpool,
        ):
            # ---- persistent SBUF
            wih_t = cpool.tile([128, LPS * WQ], FP8, tag="wih")
            whh_t = cpool.tile([128, LPS * WQ], FP8, tag="whh")
            for l in range(LPS):
                nc.sync.dma_start(wih_t[:, l * WQ:(l + 1) * WQ], wih_d[l])
                nc.sync.dma_start(whh_t[:, l * WQ:(l + 1) * WQ], whh_d[l])
            biasbc_t = cpool.tile([128, LPS * PW], BF16, tag="biasbc")
            nc.sync.dma_start(
                biasbc_t[:], biasbc_d.rearrange("l p m -> p l m"))
            h0_t = cpool.tile([128, SW], BF16, tag="h0")
            nc.sync.dma_start(h0_t[:], h0_d)
            c0_t = cpool.tile([128, SW], F32, tag="c0")
            nc.sync.dma_start(c0_t[:], c0_d)
            wo_t = cpool.tile([128, KT * VPC * 128], BF16, tag="wo")
            nc.sync.dma_start(wo_t[:], wo_d)
            bo_t = cpool.tile([128, VPC], F32, tag="bo")
            nc.sync.dma_start(bo_t[:], bo_d)
            wof_t = cpool.tile([128, KT * VMT * 128], BF16, tag="wof")
            nc.sync.dma_start(wof_t[:], wof_d)
            bof_t = cpool.tile([128, VMT], F32, tag="bof")
            nc.sync.dma_start(bof_t[:], bof_d)
            selw_t = cpool.tile([128, 4], F32, tag="selw")
            nc.sync.dma_start(selw_t[:], selw_d)
            sm_t = cpool.tile([128, NITER], F32, tag="sm")
            nc.sync.dma_start(sm_t[:], sm_d)
            ism_t = cpool.tile([128, NITER], F32, tag="ism")
            nc.sync.dma_start(ism_t[:], ism_d)
            identq_t = cpool.tile([128, 128], FP8, tag="identq")
            nc.sync.dma_start(identq_t[:], identq_d)

            cstate = [cpool.tile([128, SW], F32, tag=f"cst{l}",
                                 name=f"cst{l}") for l in range(LPS)]

            # ---- internal DRAM bounce buffers (per half-chunk, double-buffered)
            XH2 = 2 * XQ
            cc_in = [[dpool.tile([128, XH2], BF16, name=f"ccin{i}h{h}")
                      for h in range(2)] for i in range(2)]
            cc_out = [[dpool.tile([4, 128, XH2], BF16, name=f"ccout{i}h{h}")
                       for h in range(2)] for i in range(2)]

            # warm-up collective: absorbs the first-call latency of the
            # collective stack while the weight DMAs run
            wrm_in = dpool.tile([128, 16], BF16, name="wrmin")
            wrm_out = dpool.tile([4, 128, 16], BF16, name="wrmout")
            wrm_s = cpool.tile([128, 16], BF16, tag="wrm")
            nc.vector.memset(wrm_s[:], 0.0)
            nc.gpsimd.dma_start(wrm_in[:], wrm_s[:])
            nc.gpsimd.collective_compute(
                "AllGather",
                mybir.AluOpType.bypass,
                replica_groups=RG,
                ins=[wrm_in.opt()],
                outs=[wrm_out.opt()],
            )

            # emission-order chaining of ACT / DVE queues
            from concourse.tile_rust import add_dep_helper as _adh
            chain_prev = {}

            def _chain(key, r):
                raw = getattr(r, "ins", r)
                prev = chain_prev.get(key)
                if prev is not None:
                    _adh(raw, prev, sync=False, reason="stream order")
                chain_prev[key] = raw
                return r

            def wtile(base, l, m, k):
                off = (l * KT * MT + k * MT + m) * 128
                return base[:, off:off + 128]

            xh_prev = None
            ccrh_next = {}

            # ------------------------------------------------ one iteration
            for it in range(NITER):
                # 1) gathered halves were prefetched during slot it-1
                ccrh = None
                if it > 0:
                    ccrh = [ccrh_next[h] for h in range(2)]

                cdst = it - S if it >= S else C
                last_compute = it < NITER - 1

                def emit_logits(h):
                    # logits for chunk it-S, half h, from gathered stage-3 x
                    xf = ccrh[h][:, 3 * XH2:4 * XH2].rearrange(
                        "p (t k b) -> p k t b", t=2 * QS, k=KT)
                    for m in range(VPC):
                        ps = lps.tile([128, 2 * QS * BL], F32, tag="lp")
                        for k in range(KT):
                            nc.tensor.matmul(
                                ps[:],
                                lhsT=wo_t[:, (k * VPC + m) * 128:
                                          (k * VPC + m + 1) * 128],
                                rhs=xf[:, k],
                                start=(k == 0),
                                stop=(k == KT - 1),
                            )
                        lgs = gpool.tile([128, 2 * QS * BL], F32, tag="lgs")
                        _chain("act", nc.scalar.activation(
                            lgs[:], ps[:], AF.Identity,
                            bias=bo_t[:, m:m + 1]))
                        nc.sync.dma_start(
                            lg_d[m][:, cdst * TC * BL + h * 2 * QS * BL:
                                    cdst * TC * BL + (h + 1) * 2 * QS * BL],
                            lgs[:],
                        )

                # 3) x input for this slot: slot 0 loads embeddings directly;
                # later slots were prepared on DVE during slot it-1.
                if it == 0:
                    xinh = {}
                    for h in range(2):
                        x0s = xpool.tile([128, XH2], BF16, tag=f"x0s{h}",
                                         name=f"x0s{h}_0")
                        nc.scalar.dma_start(
                            x0s[:], x0_d[:, h * XH2:(h + 1) * XH2])
                        xinh[h] = x0s
                else:
                    xinh = {0: xin_next[0]}  # h1 select emitted at group 1
                xin_next = {}

                def emit_select(h, tgt_it, ccsrc, sink):
                    # DVE input select for slot tgt_it from gathered slices
                    cch = min(tgt_it, C - 1)
                    x0s = xpool.tile([128, XH2], BF16, tag=f"x0s{h}",
                                     name=f"x0s{h}_{tgt_it}")
                    nc.scalar.dma_start(
                        x0s[:],
                        x0_d[:, cch * XW + h * XH2:cch * XW + (h + 1) * XH2])
                    sel = []
                    for r in range(3):
                        tr = xpool.tile([128, XH2], BF16, tag=f"sel{h}{r}",
                                        name=f"sel{h}{r}_{tgt_it}")
                        _chain("dve", nc.vector.tensor_scalar_mul(
                            tr[:], ccsrc[:, r * XH2:(r + 1) * XH2],
                            selw_t[:, r:r + 1]))
                        sel.append(tr)
                    ta = xpool.tile([128, XH2], BF16, tag=f"sta{h}",
                                    name=f"sta{h}_{tgt_it}")
                    _chain("dve", nc.vector.tensor_add(
                        ta[:], sel[0][:], sel[1][:]))
                    tb = xpool.tile([128, XH2], BF16, tag=f"stb{h}",
                                    name=f"stb{h}_{tgt_it}")
                    _chain("dve", nc.vector.tensor_add(
                        tb[:], sel[2][:], x0s[:]))
                    xin = xpool.tile([128, XH2], BF16, tag=f"xin{h}",
                                     name=f"xin{h}_{tgt_it}")
                    _chain("dve", nc.vector.tensor_add(
                        xin[:], ta[:], tb[:]))
                    sink[h] = xin

                hprev = {}

                def emit_carry(l):
                    hp = spool.tile([128, SW], BF16, tag=f"hp{l}",
                                    name=f"hp{l}_{it}")
                    if it == 0:
                        _chain("dve", nc.vector.tensor_copy(
                            cstate[l][:], c0_t[:]))
                        _chain("dve", nc.vector.tensor_copy(hp[:], h0_t[:]))
                    else:
                        ca = spool.tile([128, SW], F32, tag="ca")
                        cb = spool.tile([128, SW], F32, tag="cb")
                        _chain("dve", nc.vector.tensor_scalar_mul(
                            ca[:], cstate[l][:], sm_t[:, it:it + 1]))
                        _chain("dve", nc.vector.tensor_scalar_mul(
                            cb[:], c0_t[:], ism_t[:, it:it + 1]))
                        _chain("dve", nc.vector.tensor_add(
                            cstate[l][:], ca[:], cb[:]))
                        ha = spool.tile([128, SW], BF16, tag="ha")
                        hb = spool.tile([128, SW], BF16, tag="hb")
                        _chain("dve", nc.vector.tensor_scalar_mul(
                            ha[:], xh_prev[(l, NQ - 1)][:, SW:2 * SW],
                            sm_t[:, it:it + 1]))
                        _chain("dve", nc.vector.tensor_scalar_mul(
                            hb[:], h0_t[:], ism_t[:, it:it + 1]))
                        _chain("dve", nc.vector.tensor_add(
                            hp[:], ha[:], hb[:]))
                    hprev[l] = hp

                # 5) LSTM: quarter-chunk wavefront across the 3 layers
                xh = {}
                for l in range(LPS):
                    for q in range(NQ):
                        xh[(l, q)] = xpool.tile(
                            [128, QS * SW], BF16, tag=f"xh{l}{q}",
                            name=f"xh{l}{q}_{it}")

                gtile = {}

                def emit_gates_mm(l, q):
                    # one PSUM tile [128, PW] = 2 banks; flat free =
                    # (m16, t2, b32); bank b holds m-tiles 8b..8b+7.
                    ps = gps.tile([128, PW], F32, tag=f"g{l}",
                                  name=f"g{l}q{q}_{it}")
                    gtile[(l, q)] = ps
                    # bias: sets has_written for the full banks
                    for bank in range(2):
                        nc.tensor.matmul(
                            ps[:, bank * GW:(bank + 1) * GW],
                            lhsT=identq_t[:],
                            rhs=biasbc_t[:, l * PW + bank * GW:
                                         l * PW + (bank + 1) * GW],
                            start=True,
                            stop=False,
                            skip_group_check=True,
                        )
                    if l == 0:
                        xsrc = xinh[q // 2][:, (q % 2) * XQ:(q % 2 + 1) * XQ]
                    else:
                        xsrc = xh[(l - 1, q)][:]
                    xk = xsrc.rearrange("p (t k b) -> p k t b",
                                        t=QS, k=KT)
                    pv = ps[:].rearrange("p (m t b) -> p m t b",
                                         m=MT, t=QS)
                    for m in range(MT):
                        for k in range(KT):
                            nc.tensor.matmul(
                                pv[:, m],
                                lhsT=wtile(wih_t, l, m, k),
                                rhs=xk[:, k],
                                start=False,
                                stop=False,
                                skip_group_check=True,
                            )

                def emit_step(l, q, t):
                    ts = q * QS + t
                    if ts == 0:
                        rhs = hprev[l][:]
                    elif t == 0:
                        rhs = xh[(l, q - 1)][:, (QS - 1) * SW:QS * SW]
                    else:
                        rhs = xh[(l, q)][:, (t - 1) * SW:t * SW]
                    ps = gtile[(l, q)]
                    for m in range(MT):
                        for k in range(KT):
                            nc.tensor.matmul(
                                ps[:, m * QS * BL + t * BL:
                                   m * QS * BL + (t + 1) * BL],
                                lhsT=wtile(whh_t, l, m, k),
                                rhs=rhs[:, k * BL:(k + 1) * BL],
                                start=False,
                                stop=(m == MT - 1 and k == KT - 1),
                                skip_group_check=True,
                            )
                    gv = ps[:].rearrange("p (m t b) -> p m t b",
                                         m=MT, t=QS)
                    sig = spool.tile([128, 3 * SW], F32, tag="sig")
                    _chain("act", nc.scalar.activation(
                        sig[:], gv[:, 0:12, t], AF.Sigmoid,
                        scale=1.0 / WSCALE))
                    tg = spool.tile([128, SW], F32, tag="tg")
                    _chain("act", nc.scalar.activation(
                        tg[:], gv[:, 12:16, t], AF.Tanh,
                        scale=1.0 / WSCALE))
                    t1 = spool.tile([128, SW], F32, tag="t1")
                    _chain("dve", nc.vector.tensor_mul(
                        t1[:], sig[:, 0:SW], tg[:]))
                    t2 = spool.tile([128, SW], F32, tag="t2")
                    _chain("dve", nc.vector.tensor_mul(
                        t2[:], sig[:, SW:2 * SW], cstate[l][:]))
                    _chain("dve", nc.vector.tensor_add(
                        cstate[l][:], t1[:], t2[:]))
                    tcn = spool.tile([128, SW], F32, tag="tcn")
                    _chain("act", nc.scalar.activation(
                        tcn[:], cstate[l][:], AF.Tanh))
                    _chain("dve", nc.vector.tensor_mul(
                        xh[(l, q)][:, t * SW:(t + 1) * SW],
                        sig[:, 2 * SW:3 * SW], tcn[:]))

                def emit_handoff(h):
                    for p in range(2):
                        nc.gpsimd.dma_start(
                            cc_in[it % 2][h][:, p * XQ:(p + 1) * XQ],
                            xh[(LPS - 1, 2 * h + p)][:])
                    nc.gpsimd.collective_compute(
                        "AllGather",
                        mybir.AluOpType.bypass,
                        replica_groups=RG,
                        ins=[cc_in[it % 2][h].opt()],
                        outs=[cc_out[it % 2][h].opt()],
                    )
                    # prefetch the gathered result for slot it+1 (sync DGE
                    # queue carries only these, so the AG-done wait cannot
                    # block anything else)
                    nxt = rpool.tile([128, 4 * XH2], BF16, tag=f"ccr{h}",
                                     name=f"ccr{h}_{it + 1}")
                    for r in range(4):
                        nc.sync.dma_start(
                            nxt[:, r * XH2:(r + 1) * XH2],
                            cc_out[it % 2][h][r])
                    ccrh_next[h] = nxt

                # wavefront: group g holds tasks (l, q) with l + q == g
                for gidx in range(LPS + NQ - 1):
                    grp = [(l, gidx - l) for l in range(LPS)
                           if 0 <= gidx - l < NQ]
                    for (l, q) in grp:
                        if q == 0:
                            emit_carry(l)
                    # h1 input select for THIS slot: its prefetch lands a few
                    # us into the slot; first consumer is (0,2) at group 2
                    if it >= 1 and gidx == 1:
                        emit_select(1, it, ccrh[1], xinh)
                    if it >= 1 and gidx == 1:
                        emit_logits(0)
                    # interleave so task B's gates MMs hide task A's step-0
                    # recurrence chain on the PE
                    for (l, q) in grp:
                        emit_gates_mm(l, q)
                        emit_step(l, q, 0)
                    # h1 logits fill the rec-chain stall in the singleton
                    # drain group; its (slot it-1) AllGather completed
                    # ~30us into this slot.
                    if it >= 1 and gidx == LPS + 2:
                        emit_logits(1)
                    for (l, q) in grp:
                        emit_step(l, q, 1)
                    # the final slot's stage outputs have no consumer: the
                    # last chunk's logits are computed locally below
                    if gidx in (LPS, LPS + 2) and last_compute:
                        emit_handoff((gidx - LPS) // 2)

                if not last_compute:
                    # final chunk's full-V logits from local stage-3 xh
                    # (zero weights and a dead store on non-stage-3 cores)
                    for q in range(NQ):
                        xf = xh[(LPS - 1, q)][:].rearrange(
                            "p (t k b) -> p k t b", t=QS, k=KT)
                        for m in range(VMT):
                            ps = lps.tile([128, QS * BL], F32, tag="lp")
                            for k in range(KT):
                                nc.tensor.matmul(
                                    ps[:],
                                    lhsT=wof_t[:, (k * VMT + m) * 128:
                                               (k * VMT + m + 1) * 128],
                                    rhs=xf[:, k],
                                    start=(k == 0),
                                    stop=(k == KT - 1),
                                )
                            lgs = gpool.tile([128, QS * BL], F32, tag="lgs")
                            _chain("act", nc.scalar.activation(
                                lgs[:], ps[:], AF.Identity,
                                bias=bof_t[:, m:m + 1]))
                            nc.sync.dma_start(
                                lgf_d[m][:, q * QS * BL:(q + 1) * QS * BL],
                                lgs[:],
                            )

                # h0 input select for slot it+1 at the tail of this slot's
                # DVE chain: its prefetch completed ~85% through this slot
                if last_compute:
                    emit_select(0, it + 1, ccrh_next[0], xin_next)

                xh_prev = xh

    _split_excess_waits(nc)
    return nc


# ---------------------------------------------------------------- interface
_CACHE = {}


def _get_program():
    if "nc" not in _CACHE:
        _CACHE["nc"] = _build_program()
    return _CACHE["nc"]


def run(trace=False, **inputs):
    nc = _get_program()
    in_maps = [_prep_core(inputs, ci) for ci in range(NCORES)]
    res = run_bass_kernel_spmd(nc, in_maps, list(range(NCORES)), trace=trace)

    out = np.empty((B, T, V), np.float32)
    for g in range(NGRP):
        for v in range(VMT):
            core = g * S + v // VPC
            lt = np.asarray(res.results[core]["lg"][v % VPC], np.float32)
            lo = v * 128
            hi = min(lo + 128, V)
            # lt[p, c*TC*BL + t*BL + b] -> out[g*BL+b, c*TC+t, lo+p]
            lt = lt[:, :C * TC * BL].reshape(128, C, TC, BL)
            out[g * BL:(g + 1) * BL, :, lo:hi] = (
                lt[:hi - lo].transpose(3, 1, 2, 0).reshape(BL, T, hi - lo)
            )
        # final chunk comes from the stage-3 core's local full-V logits
        lf = np.asarray(res.results[g * S + S - 1]["lgf"], np.float32)
        lf = lf.reshape(VMT, 128, TC, BL)[:, :, :, :].transpose(3, 2, 0, 1)
        out[g * BL:(g + 1) * BL, (C - 1) * TC:, :] = (
            lf.reshape(BL, TC, VP)[:, :, :V]
        )
    return out, res


def kernel(**inputs):
    return run(trace=False, **inputs)[0]


# revision 41
# speedup vs baseline: 1.0015x; 1.0015x over previous
"""Pipelined Trainium2 Bass kernel for the 12-layer LSTM decoder (v2).

Topology: 4 pipeline stages x 2 batch groups over 8 cores.
  core ci: stage s = ci % 4 (layers 3s..3s+2, weights SBUF-resident),
           group g = ci // 4 (batch rows 32g..32g+31).
Chunked pipeline over T: C = T/TC chunks; iteration k is a global slot;
stage s computes chunk c = k - s.  Uniform SPMD: warm-up / drain slots
compute on finite garbage (zero inputs), selected away by per-core 0/1
masks carried in the input map.

v2 changes vs baseline:
  - gates accumulate directly in PSUM: per (layer, quarter) one 2-bank
    PSUM tile, written by bias ident-MMs (start=True), then input MMs
    (W_ih@x), then per-step recurrence MMs (W_hh@h); sigma/tanh read
    PSUM directly.  Kills the per-(m) bias ACTs and per-step ident MMs.
  - W_ih/W_hh stored fp8e3m4 scaled x32 (stationary operand only; FWL
    loads 4 elem/cycle -> half the LDWEIGHTS cost of bf16).  Descale by
    1/32 via the free ACT scale port at sigma/tanh.  W_out stays bf16.
  - AllGather handoff at quarter-chunk granularity (4/slot) so the next
    slot's stage input is ready before this slot ends.
  - x-select and state-carry muxes moved from ACT to DVE tensor_scalar.
"""

import os
import sys

import numpy as np

for _p in ("/opt/trn_rl_repo",):
    if os.path.isdir(_p) and _p not in sys.path:
        sys.path.insert(0, _p)

import ml_dtypes  # noqa: E402

import concourse.bass as bass  # noqa: E402
import concourse.mybir as mybir  # noqa: E402
import concourse.tile as tile  # noqa: E402
from concourse.bass_utils import run_bass_kernel_spmd  # noqa: E402

# ---------------------------------------------------------------- constants
L = 12
B = 64
T = 128
E = 512
H = 512
V = 1000
BOS_ID = 1

NCORES = 8
S = 4                     # pipeline stages
NGRP = 2                  # batch groups
LPS = L // S              # layers per stage = 3
BL = B // NGRP            # batch rows per core = 32
KT = H // 128             # K tiles per 512 contraction = 4
MT = (4 * H) // 128       # M tiles over gate rows = 16
VP = 1024
VMT = VP // 128           # = 8
VPC = VMT // S            # V-tiles per core = 2
TC = 8                    # timesteps per chunk
C = T // TC               # chunks = 16
NITER = C + S - 1         # pipeline slots = 19 (last chunk's logits are
                          # computed locally on stage-3 cores, no drain slot)
NQ = 4                    # quarters per chunk
QS = TC // NQ             # steps per quarter = 2

SW = KT * BL              # per-step h width = 128
GW = MT * BL              # per-step gate width = 512
PW = QS * GW              # psum tile per (l, quarter) = 1024 (2 banks)
XW = TC * KT * BL         # x-chunk width = 1024
XQ = QS * KT * BL         # quarter x width = 256
WSCALE = 32.0             # fp8 weight pre-scale

F32 = mybir.dt.float32
BF16 = mybir.dt.bfloat16
FP8 = mybir.dt.float8e3
BF16_NP = ml_dtypes.bfloat16
FP8_NP = ml_dtypes.float8_e3m4
AF = mybir.ActivationFunctionType
ALU = mybir.AluOpType


# ------------------------------------------------------------- host packing
def _pack_w_stack(w_stack, mt, np_dtype):
    w = np.asarray(w_stack, np.float32)
    squeeze = w.ndim == 2
    if squeeze:
        w = w[None]
    n = w.shape[0]
    kt = w.shape[2] // 128
    out = (
        w.reshape(n, mt, 128, kt, 128)
        .transpose(0, 4, 3, 1, 2)
        .reshape(n, 128, kt * mt * 128)
        .astype(np_dtype)
    )
    return out[0] if squeeze else out


_GATE_PERM = np.concatenate(
    [
        np.arange(0, H),
        np.arange(H, 2 * H),
        np.arange(3 * H, 4 * H),
        np.arange(2 * H, 3 * H),
    ]
)


def _prep_core(inputs, ci):
    st = ci % S
    g = ci // S
    lsl = slice(st * LPS, (st + 1) * LPS)
    bs = slice(g * BL, (g + 1) * BL)

    W_ih = np.asarray(inputs["W_ih"], np.float32)[lsl][:, _GATE_PERM, :]
    W_hh = np.asarray(inputs["W_hh"], np.float32)[lsl][:, _GATE_PERM, :]
    b = (
        np.asarray(inputs["b_ih"], np.float32)
        + np.asarray(inputs["b_hh"], np.float32)
    )[lsl][:, _GATE_PERM]
    wih = _pack_w_stack(W_ih * WSCALE, MT, FP8_NP)            # [3,128,8192]
    whh = _pack_w_stack(W_hh * WSCALE, MT, FP8_NP)

    # bias broadcast tile matching the psum bank layout:
    # per (l): [128, PW] with flat free = (m16, t2, b32); value = WSCALE*b
    bm = (b * WSCALE).reshape(LPS, MT, 128)                   # [3, m, p]
    biasbc = np.broadcast_to(
        bm.transpose(0, 2, 1)[:, :, :, None, None],           # [3,p,m,1,1]
        (LPS, 128, MT, QS, BL),
    ).reshape(LPS, 128, PW).astype(BF16_NP)

    # x0 embeddings (stage 0 only; zeros elsewhere)
    if st == 0:
        tok = np.asarray(inputs["target_token_ids"])
        tokens = np.concatenate(
            [np.full((B, 1), BOS_ID, tok.dtype), tok[:, :-1]], axis=1
        )
        emb = np.asarray(inputs["embed_table"], np.float32)
        e = emb[tokens[bs]]                                   # [32, T, 512]
        x0 = (
            e.reshape(BL, T, KT, 128)
            .transpose(3, 1, 2, 0)
            .reshape(128, T * KT * BL)
            .astype(BF16_NP)
        )
    else:
        x0 = np.zeros((128, T * KT * BL), BF16_NP)

    h0 = np.asarray(inputs["h0"], np.float32)[bs]
    c0 = np.asarray(inputs["c0"], np.float32)[bs]
    h0T = h0.reshape(BL, KT, 128).transpose(2, 1, 0).reshape(128, SW)
    c0T = c0.reshape(BL, KT, 128).transpose(2, 1, 0).reshape(128, SW)

    # output projection slice: V-tiles (2*st, 2*st+1)
    Wo = np.zeros((VP, H), np.float32)
    Wo[:V] = np.asarray(inputs["W_out"], np.float32)
    wo_rows = Wo[st * VPC * 128:(st + 1) * VPC * 128]         # [256, 512]
    wouts = _pack_w_stack(wo_rows, VPC, BF16_NP)              # [128, KT*2*128]
    bo = np.zeros((VP,), np.float32)
    bo[:V] = np.asarray(inputs["b_out"], np.float32)
    bouts = bo.reshape(VMT, 128).T[:, st * VPC:(st + 1) * VPC].copy()

    # full output projection, used only by stage-3 cores to produce the
    # final chunk's logits locally (no AllGather round trip)
    if st == S - 1:
        wof = _pack_w_stack(Wo, VMT, BF16_NP)                 # [128, KT*8*128]
        bof = bo.reshape(VMT, 128).T.copy()                   # [128, 8]
    else:
        wof = np.zeros((128, KT * VMT * 128), BF16_NP)
        bof = np.zeros((128, VMT), np.float32)

    # role masks
    selw = np.zeros((128, 4), np.float32)
    if st > 0:
        selw[:, st - 1] = 1.0
    smask = np.zeros((128, NITER), np.float32)
    smask[:, st + 1:] = 1.0                                   # carry iff k > st
    ismask = 1.0 - smask

    return {
        "wih": np.ascontiguousarray(wih),
        "whh": np.ascontiguousarray(whh),
        "biasbc": np.ascontiguousarray(biasbc),
        "x0": np.ascontiguousarray(x0),
        "h0T": np.ascontiguousarray(h0T.astype(BF16_NP)),
        "c0T": np.ascontiguousarray(c0T.astype(np.float32)),
        "wouts": np.ascontiguousarray(wouts),
        "bouts": np.ascontiguousarray(bouts),
        "wof": np.ascontiguousarray(wof),
        "bof": np.ascontiguousarray(bof),
        "selw": selw,
        "smask": smask,
        "ismask": ismask,
        "identq": np.ascontiguousarray(np.eye(128, dtype=FP8_NP)),
    }


# ------------------------------------------------------------ device kernel
_WAIT_LIMITS = {}


def _split_excess_waits(nc, default_limit=1):
    import bass_rust as _br

    n_split = 0
    for fn in nc.m.functions:
        for bb in fn.blocks:
            insts = list(bb.instructions)
            out = []
            changed = False
            for inst in insts:
                tname = type(inst).__name__
                limit = _WAIT_LIMITS.get(tname, default_limit)
                si = getattr(inst, "sync_info", None)
                ow = list(si.on_wait) if si is not None and si.on_wait else []
                if limit is not None and len(ow) > limit:
                    keep = ow[-limit:] if limit else []
                    rest = ow[:len(ow) - limit]
                    for j, w in enumerate(rest):
                        ev = mybir.InstEventSemaphore(
                            name=f"{inst.name}_wsplit{j}"
                        )
                        ev.engine = inst.engine
                        ev.sync_info = _br.SyncInfo(on_wait=[w], on_update=[])
                        out.append(ev)
                        n_split += 1
                    inst.sync_info = _br.SyncInfo(
                        on_wait=keep, on_update=list(si.on_update or [])
                    )
                    changed = True
                out.append(inst)
            if changed:
                bb.instructions = out
    return n_split


def _build_program():
    nc = bass.Bass(
        "TRN2", target_bir_lowering=False, debug=False, enable_asserts=False
    )

    WQ = KT * MT * 128
    wih_d = nc.dram_tensor("wih", [LPS, 128, WQ], FP8,
                           kind="ExternalInput").ap()
    whh_d = nc.dram_tensor("whh", [LPS, 128, WQ], FP8,
                           kind="ExternalInput").ap()
    biasbc_d = nc.dram_tensor("biasbc", [LPS, 128, PW], BF16,
                              kind="ExternalInput").ap()
    x0_d = nc.dram_tensor("x0", [128, T * KT * BL], BF16,
                          kind="ExternalInput").ap()
    h0_d = nc.dram_tensor("h0T", [128, SW], BF16, kind="ExternalInput").ap()
    c0_d = nc.dram_tensor("c0T", [128, SW], F32, kind="ExternalInput").ap()
    wo_d = nc.dram_tensor("wouts", [128, KT * VPC * 128], BF16,
                          kind="ExternalInput").ap()
    bo_d = nc.dram_tensor("bouts", [128, VPC], F32, kind="ExternalInput").ap()
    wof_d = nc.dram_tensor("wof", [128, KT * VMT * 128], BF16,
                           kind="ExternalInput").ap()
    bof_d = nc.dram_tensor("bof", [128, VMT], F32, kind="ExternalInput").ap()
    selw_d = nc.dram_tensor("selw", [128, 4], F32, kind="ExternalInput").ap()
    sm_d = nc.dram_tensor("smask", [128, NITER], F32,
                          kind="ExternalInput").ap()
    ism_d = nc.dram_tensor("ismask", [128, NITER], F32,
                           kind="ExternalInput").ap()
    identq_d = nc.dram_tensor("identq", [128, 128], FP8,
                              kind="ExternalInput").ap()
    lg_d = nc.dram_tensor("lg", [VPC, 128, (C + 1) * TC * BL], F32,
                          kind="ExternalOutput").ap()
    lgf_d = nc.dram_tensor("lgf", [VMT, 128, TC * BL], F32,
                           kind="ExternalOutput").ap()

    RG = [[0, 1, 2, 3], [4, 5, 6, 7]]

    with tile.TileContext(nc) as tc:
        with (
            tc.tile_pool(name="const", bufs=1) as cpool,
            tc.tile_pool(name="xpool", bufs=2) as xpool,
            tc.tile_pool(name="ccr", bufs=2) as rpool,
            tc.tile_pool(name="spool", bufs=8) as spool,
            tc.tile_pool(name="gpool", bufs=6) as gpool,
            tc.tile_pool(name="gps", bufs=1, space="PSUM") as gps,
            tc.tile_pool(name="lps", bufs=2, space="PSUM") as lps,
            tc.tile_pool(name="dram", bufs=2, space="DRAM") as dpool,
        ):
            # ---- persistent SBUF
            wih_t = cpool.tile([128, LPS * WQ], FP8, tag="wih")
            whh_t = cpool.tile([128, LPS * WQ], FP8, tag="whh")
            for l in range(LPS):
                nc.sync.dma_start(wih_t[:, l * WQ:(l + 1) * WQ], wih_d[l])
                nc.sync.dma_start(whh_t[:, l * WQ:(l + 1) * WQ], whh_d[l])
            biasbc_t = cpool.tile([128, LPS * PW], BF16, tag="biasbc")
            nc.sync.dma_start(
                biasbc_t[:], biasbc_d.rearrange("l p m -> p l m"))
            h0_t = cpool.tile([128, SW], BF16, tag="h0")
            nc.sync.dma_start(h0_t[:], h0_d)
            c0_t = cpool.tile([128, SW], F32, tag="c0")
            nc.sync.dma_start(c0_t[:], c0_d)
            wo_t = cpool.tile([128, KT * VPC * 128], BF16, tag="wo")
            nc.sync.dma_start(wo_t[:], wo_d)
            bo_t = cpool.tile([128, VPC], F32, tag="bo")
            nc.sync.dma_start(bo_t[:], bo_d)
            wof_t = cpool.tile([128, KT * VMT * 128], BF16, tag="wof")
            nc.sync.dma_start(wof_t[:], wof_d)
            bof_t = cpool.tile([128, VMT], F32, tag="bof")
            nc.sync.dma_start(bof_t[:], bof_d)
            selw_t = cpool.tile([128, 4], F32, tag="selw")
            nc.sync.dma_start(selw_t[:], selw_d)
            sm_t = cpool.tile([128, NITER], F32, tag="sm")
            nc.sync.dma_start(sm_t[:], sm_d)
            ism_t = cpool.tile([128, NITER], F32, tag="ism")
            nc.sync.dma_start(ism_t[:], ism_d)
            identq_t = cpool.tile([128, 128], FP8, tag="identq")
            nc.sync.dma_start(identq_t[:], identq_d)

            cstate = [cpool.tile([128, SW], F32, tag=f"cst{l}",
                                 name=f"cst{l}") for l in range(LPS)]

            # ---- internal DRAM bounce buffers (per half-chunk, double-buffered)
            XH2 = 2 * XQ
            cc_in = [[dpool.tile([128, XH2], BF16, name=f"ccin{i}h{h}")
                      for h in range(2)] for i in range(2)]
            cc_out = [[dpool.tile([4, 128, XH2], BF16, name=f"ccout{i}h{h}")
                       for h in range(2)] for i in range(2)]

            # warm-up collective: absorbs the first-call latency of the
            # collective stack while the weight DMAs run
            wrm_in = dpool.tile([128, 16], BF16, name="wrmin")
            wrm_out = dpool.tile([4, 128, 16], BF16, name="wrmout")
            wrm_s = cpool.tile([128, 16], BF16, tag="wrm")
            nc.vector.memset(wrm_s[:], 0.0)
            nc.gpsimd.dma_start(wrm_in[:], wrm_s[:])
            nc.gpsimd.collective_compute(
                "AllGather",
                mybir.AluOpType.bypass,
                replica_groups=RG,
                ins=[wrm_in.opt()],
                outs=[wrm_out.opt()],
            )

            # emission-order chaining of ACT / DVE queues
            from concourse.tile_rust import add_dep_helper as _adh
            chain_prev = {}

            def _chain(key, r):
                raw = getattr(r, "ins", r)
                prev = chain_prev.get(key)
                if prev is not None:
                    _adh(raw, prev, sync=False, reason="stream order")
                chain_prev[key] = raw
                return r

            def wtile(base, l, m, k):
                off = (l * KT * MT + k * MT + m) * 128
                return base[:, off:off + 128]

            xh_prev = None
            ccrh_next = {}

            # ------------------------------------------------ one iteration
            for it in range(NITER):
                # 1) gathered halves were prefetched during slot it-1
                ccrh = None
                if it > 0:
                    ccrh = [ccrh_next[h] for h in range(2)]

                cdst = it - S if it >= S else C
                last_compute = it < NITER - 1

                def emit_logits(h, ms=range(VPC)):
                    # logits for chunk it-S, half h, from gathered stage-3 x
                    xf = ccrh[h][:, 3 * XH2:4 * XH2].rearrange(
                        "p (t k b) -> p k t b", t=2 * QS, k=KT)
                    for m in ms:
                        ps = lps.tile([128, 2 * QS * BL], F32, tag="lp")
                        for k in range(KT):
                            nc.tensor.matmul(
                                ps[:],
                                lhsT=wo_t[:, (k * VPC + m) * 128:
                                          (k * VPC + m + 1) * 128],
                                rhs=xf[:, k],
                                start=(k == 0),
                                stop=(k == KT - 1),
                            )
                        lgs = gpool.tile([128, 2 * QS * BL], F32, tag="lgs")
                        _chain("act", nc.scalar.activation(
                            lgs[:], ps[:], AF.Identity,
                            bias=bo_t[:, m:m + 1]))
                        nc.sync.dma_start(
                            lg_d[m][:, cdst * TC * BL + h * 2 * QS * BL:
                                    cdst * TC * BL + (h + 1) * 2 * QS * BL],
                            lgs[:],
                        )

                # 3) x input for this slot: slot 0 loads embeddings directly;
                # later slots were prepared on DVE during slot it-1.
                if it == 0:
                    xinh = {}
                    for h in range(2):
                        x0s = xpool.tile([128, XH2], BF16, tag=f"x0s{h}",
                                         name=f"x0s{h}_0")
                        nc.scalar.dma_start(
                            x0s[:], x0_d[:, h * XH2:(h + 1) * XH2])
                        xinh[h] = x0s
                else:
                    xinh = {0: xin_next[0]}  # h1 select emitted at group 1
                xin_next = {}

                def emit_select(h, tgt_it, ccsrc, sink):
                    # DVE input select for slot tgt_it from gathered slices
                    cch = min(tgt_it, C - 1)
                    x0s = xpool.tile([128, XH2], BF16, tag=f"x0s{h}",
                                     name=f"x0s{h}_{tgt_it}")
                    nc.scalar.dma_start(
                        x0s[:],
                        x0_d[:, cch * XW + h * XH2:cch * XW + (h + 1) * XH2])
                    sel = []
                    for r in range(3):
                        tr = xpool.tile([128, XH2], BF16, tag=f"sel{h}{r}",
                                        name=f"sel{h}{r}_{tgt_it}")
                        _chain("dve", nc.vector.tensor_scalar_mul(
                            tr[:], ccsrc[:, r * XH2:(r + 1) * XH2],
                            selw_t[:, r:r + 1]))
                        sel.append(tr)
                    ta = xpool.tile([128, XH2], BF16, tag=f"sta{h}",
                                    name=f"sta{h}_{tgt_it}")
                    _chain("dve", nc.vector.tensor_add(
                        ta[:], sel[0][:], sel[1][:]))
                    tb = xpool.tile([128, XH2], BF16, tag=f"stb{h}",
                                    name=f"stb{h}_{tgt_it}")
                    _chain("dve", nc.vector.tensor_add(
                        tb[:], sel[2][:], x0s[:]))
                    xin = xpool.tile([128, XH2], BF16, tag=f"xin{h}",
                                     name=f"xin{h}_{tgt_it}")
                    _chain("dve", nc.vector.tensor_add(
                        xin[:], ta[:], tb[:]))
                    sink[h] = xin

                hprev = {}

                def emit_carry(l):
                    hp = spool.tile([128, SW], BF16, tag=f"hp{l}",
                                    name=f"hp{l}_{it}")
                    if it == 0:
                        _chain("dve", nc.vector.tensor_copy(
                            cstate[l][:], c0_t[:]))
                        _chain("dve", nc.vector.tensor_copy(hp[:], h0_t[:]))
                    else:
                        ca = spool.tile([128, SW], F32, tag="ca")
                        cb = spool.tile([128, SW], F32, tag="cb")
                        _chain("dve", nc.vector.tensor_scalar_mul(
                            ca[:], cstate[l][:], sm_t[:, it:it + 1]))
                        _chain("dve", nc.vector.tensor_scalar_mul(
                            cb[:], c0_t[:], ism_t[:, it:it + 1]))
                        _chain("dve", nc.vector.tensor_add(
                            cstate[l][:], ca[:], cb[:]))
                        ha = spool.tile([128, SW], BF16, tag="ha")
                        hb = spool.tile([128, SW], BF16, tag="hb")
                        _chain("dve", nc.vector.tensor_scalar_mul(
                            ha[:], xh_prev[(l, NQ - 1)][:, SW:2 * SW],
                            sm_t[:, it:it + 1]))
                        _chain("dve", nc.vector.tensor_scalar_mul(
                            hb[:], h0_t[:], ism_t[:, it:it + 1]))
                        _chain("dve", nc.vector.tensor_add(
                            hp[:], ha[:], hb[:]))
                    hprev[l] = hp

                # 5) LSTM: quarter-chunk wavefront across the 3 layers
                xh = {}
                for l in range(LPS):
                    for q in range(NQ):
                        xh[(l, q)] = xpool.tile(
                            [128, QS * SW], BF16, tag=f"xh{l}{q}",
                            name=f"xh{l}{q}_{it}")

                gtile = {}

                def emit_gates_mm(l, q):
                    # one PSUM tile [128, PW] = 2 banks; flat free =
                    # (m16, t2, b32); bank b holds m-tiles 8b..8b+7.
                    ps = gps.tile([128, PW], F32, tag=f"g{l}",
                                  name=f"g{l}q{q}_{it}")
                    gtile[(l, q)] = ps
                    # bias: sets has_written for the full banks
                    for bank in range(2):
                        nc.tensor.matmul(
                            ps[:, bank * GW:(bank + 1) * GW],
                            lhsT=identq_t[:],
                            rhs=biasbc_t[:, l * PW + bank * GW:
                                         l * PW + (bank + 1) * GW],
                            start=True,
                            stop=False,
                            skip_group_check=True,
                        )
                    if l == 0:
                        xsrc = xinh[q // 2][:, (q % 2) * XQ:(q % 2 + 1) * XQ]
                    else:
                        xsrc = xh[(l - 1, q)][:]
                    xk = xsrc.rearrange("p (t k b) -> p k t b",
                                        t=QS, k=KT)
                    pv = ps[:].rearrange("p (m t b) -> p m t b",
                                         m=MT, t=QS)
                    for m in range(MT):
                        for k in range(KT):
                            nc.tensor.matmul(
                                pv[:, m],
                                lhsT=wtile(wih_t, l, m, k),
                                rhs=xk[:, k],
                                start=False,
                                stop=False,
                                skip_group_check=True,
                            )

                def emit_step(l, q, t):
                    ts = q * QS + t
                    if ts == 0:
                        rhs = hprev[l][:]
                    elif t == 0:
                        rhs = xh[(l, q - 1)][:, (QS - 1) * SW:QS * SW]
                    else:
                        rhs = xh[(l, q)][:, (t - 1) * SW:t * SW]
                    ps = gtile[(l, q)]
                    for m in range(MT):
                        for k in range(KT):
                            nc.tensor.matmul(
                                ps[:, m * QS * BL + t * BL:
                                   m * QS * BL + (t + 1) * BL],
                                lhsT=wtile(whh_t, l, m, k),
                                rhs=rhs[:, k * BL:(k + 1) * BL],
                                start=False,
                                stop=(m == MT - 1 and k == KT - 1),
                                skip_group_check=True,
                            )
                    gv = ps[:].rearrange("p (m t b) -> p m t b",
                                         m=MT, t=QS)
                    sig = spool.tile([128, 3 * SW], F32, tag="sig")
                    _chain("act", nc.scalar.activation(
                        sig[:], gv[:, 0:12, t], AF.Sigmoid,
                        scale=1.0 / WSCALE))
                    tg = spool.tile([128, SW], F32, tag="tg")
                    _chain("act", nc.scalar.activation(
                        tg[:], gv[:, 12:16, t], AF.Tanh,
                        scale=1.0 / WSCALE))
                    t1 = spool.tile([128, SW], F32, tag="t1")
                    _chain("dve", nc.vector.tensor_mul(
                        t1[:], sig[:, 0:SW], tg[:]))
                    t2 = spool.tile([128, SW], F32, tag="t2")
                    _chain("dve", nc.vector.tensor_mul(
                        t2[:], sig[:, SW:2 * SW], cstate[l][:]))
                    _chain("dve", nc.vector.tensor_add(
                        cstate[l][:], t1[:], t2[:]))
                    tcn = spool.tile([128, SW], F32, tag="tcn")
                    _chain("act", nc.scalar.activation(
                        tcn[:], cstate[l][:], AF.Tanh))
                    _chain("dve", nc.vector.tensor_mul(
                        xh[(l, q)][:, t * SW:(t + 1) * SW],
                        sig[:, 2 * SW:3 * SW], tcn[:]))

                def emit_handoff(h):
                    for p in range(2):
                        nc.gpsimd.dma_start(
                            cc_in[it % 2][h][:, p * XQ:(p + 1) * XQ],
                            xh[(LPS - 1, 2 * h + p)][:])
                    nc.gpsimd.collective_compute(
                        "AllGather",
                        mybir.AluOpType.bypass,
                        replica_groups=RG,
                        ins=[cc_in[it % 2][h].opt()],
                        outs=[cc_out[it % 2][h].opt()],
                    )
                    # prefetch the gathered result for slot it+1 (sync DGE
                    # queue carries only these, so the AG-done wait cannot
                    # block anything else)
                    nxt = rpool.tile([128, 4 * XH2], BF16, tag=f"ccr{h}",
                                     name=f"ccr{h}_{it + 1}")
                    for r in range(4):
                        nc.sync.dma_start(
                            nxt[:, r * XH2:(r + 1) * XH2],
                            cc_out[it % 2][h][r])
                    ccrh_next[h] = nxt

                # wavefront: group g holds tasks (l, q) with l + q == g
                for gidx in range(LPS + NQ - 1):
                    grp = [(l, gidx - l) for l in range(LPS)
                           if 0 <= gidx - l < NQ]
                    for (l, q) in grp:
                        if q == 0:
                            emit_carry(l)
                    # h1 input select for THIS slot: its prefetch lands a few
                    # us into the slot; first consumer is (0,2) at group 2
                    if it >= 1 and gidx == 1:
                        emit_select(1, it, ccrh[1], xinh)
                    # interleave so task B's gates MMs hide task A's step-0
                    # recurrence chain on the PE
                    for (l, q) in grp:
                        emit_gates_mm(l, q)
                        emit_step(l, q, 0)
                    # logits split per V-tile as rec-chain-stall filler in the
                    # thin ramp/drain groups (h0's gather data is ready before
                    # slot start, h1's ~30us in)
                    if it >= 1 and gidx in (0, 1, LPS + 1, LPS + 2):
                        emit_logits(int(gidx >= LPS + 1), [gidx % 2])
                    for (l, q) in grp:
                        emit_step(l, q, 1)
                    # the final slot's stage outputs have no consumer: the
                    # last chunk's logits are computed locally below
                    if gidx in (LPS, LPS + 2) and last_compute:
                        emit_handoff((gidx - LPS) // 2)

                if not last_compute:
                    # final chunk's full-V logits from local stage-3 xh
                    # (zero weights and a dead store on non-stage-3 cores)
                    for q in range(NQ):
                        xf = xh[(LPS - 1, q)][:].rearrange(
                            "p (t k b) -> p k t b", t=QS, k=KT)
                        for m in range(VMT):
                            ps = lps.tile([128, QS * BL], F32, tag="lp")
                            for k in range(KT):
                                nc.tensor.matmul(
                                    ps[:],
                                    lhsT=wof_t[:, (k * VMT + m) * 128:
                                               (k * VMT + m + 1) * 128],
                                    rhs=xf[:, k],
                                    start=(k == 0),
                                    stop=(k == KT - 1),
                                )
                            lgs = gpool.tile([128, QS * BL], F32, tag="lgs")
                            _chain("act", nc.scalar.activation(
                                lgs[:], ps[:], AF.Identity,
                                bias=bof_t[:, m:m + 1]))
                            nc.sync.dma_start(
                                lgf_d[m][:, q * QS * BL:(q + 1) * QS * BL],
                                lgs[:],
                            )

                # h0 input select for slot it+1 at the tail of this slot's
                # DVE chain: its prefetch completed ~85% through this slot
                if last_compute:
                    emit_select(0, it + 1, ccrh_next[0], xin_next)

                xh_prev = xh

    _split_excess_waits(nc)
    return nc


# ---------------------------------------------------------------- interface
_CACHE = {}


def _get_program():
    if "nc" not in _CACHE:
        _CACHE["nc"] = _build_program()
    return _CACHE["nc"]


def run(trace=False, **inputs):
    nc = _get_program()
    in_maps = [_prep_core(inputs, ci) for ci in range(NCORES)]
    res = run_bass_kernel_spmd(nc, in_maps, list(range(NCORES)), trace=trace)

    out = np.empty((B, T, V), np.float32)
    for g in range(NGRP):
        for v in range(VMT):
            core = g * S + v // VPC
            lt = np.asarray(res.results[core]["lg"][v % VPC], np.float32)
            lo = v * 128
            hi = min(lo + 128, V)
            # lt[p, c*TC*BL + t*BL + b] -> out[g*BL+b, c*TC+t, lo+p]
            lt = lt[:, :C * TC * BL].reshape(128, C, TC, BL)
            out[g * BL:(g + 1) * BL, :, lo:hi] = (
                lt[:hi - lo].transpose(3, 1, 2, 0).reshape(BL, T, hi - lo)
            )
        # final chunk comes from the stage-3 core's local full-V logits
        lf = np.asarray(res.results[g * S + S - 1]["lgf"], np.float32)
        lf = lf.reshape(VMT, 128, TC, BL)[:, :, :, :].transpose(3, 2, 0, 1)
        out[g * BL:(g + 1) * BL, (C - 1) * TC:, :] = (
            lf.reshape(BL, TC, VP)[:, :, :V]
        )
    return out, res


def kernel(**inputs):
    return run(trace=False, **inputs)[0]
